# revision 7
# baseline (speedup 1.0000x reference)
"""Trainium2 Bass kernel for the CoverageMechanism (repeat-penalty) problem.

Reference semantics: for logits [B=4, S=512, V=32000] and generated_tokens
[B, S], the output is

    out[b, i, v] = logits[b, i, v] - 0.3 * #{j in [i-4, i) : tokens[b, j] == v}

for i >= 4, and out = logits for i < 4.  That is a 262 MB identity map plus
an extremely sparse update: each (b, i) row of 32000 floats gets at most 4
elements decremented.

Strategy (8 NeuronCores):
  - Flatten (b, i) to 2048 rows, shard 256 rows per core (window never
    crosses a batch row boundary; host has all tokens so no halo exchange).
  - Host preprocesses the 8 KB token tensor into per-core DMA scatter-add
    metadata (block indices + 64-float payload vectors holding -0.3*count).
  - Device (primary, in-place): the kernel output DRAM tensor is aliased to
    the donated logits input buffer (XLA input/output aliasing through
    bass2jax's `lowering_input_output_aliases`), so the 32.75 MB per-core
    bulk copy disappears entirely.  The kernel is then just: load ~280 KB of
    scatter metadata to SBUF, generate the SWDGE descriptors (GPSIMD Q7),
    and fire 4 dma_scatter_add calls (CCE read-modify-write add) that apply
    the sparse penalties in place on the logits buffer.  Each scatter covers
    64 rows (32000 blocks of 64 floats -> block ids fit int16) and all 256
    block indices within a scatter are unique, so the CCE RMW has no
    same-address races.
  - kernel() verifies the in-place result against a host-computed exact
    check on every updated element (plus untouched samples); if the aliasing
    path is ever unavailable it falls back to the previous proven
    copy-then-scatter kernel (DRAM->DRAM copy overlapped with scatters).

Both program builders accept `reps`: the same kernel body unrolled N times
with full cross-rep synchronization.  A fresh-metadata load is included in
every rep, so one rep == one complete kernel invocation; harnesses use the
wall-time slope over `reps` to measure per-invocation device time without
the multi-ms host->device dispatch overhead of the axon relay.
"""

import numpy as np

import concourse.bass as bass  # noqa: F401  (kept for parity with bacc deps)
import concourse.bacc as bacc
import concourse.mybir as mybir

B, S, V = 4, 512, 32000
M = 4                      # sliding window length
W = np.float32(0.3)        # penalty weight
NCORES = 8
R = (B * S) // NCORES      # 256 rows per core
N = R * V                  # 8_192_000 f32 per core
NWIN = 4                   # scatter windows per core
WROWS = R // NWIN          # 64 rows per window
K = WROWS * M              # 256 scatter slots per window
ES = 64                    # scatter elem_size (64 f32 = 256 B blocks)
BPR = V // ES              # 500 blocks per row
CHUNK = N // NWIN          # 2_048_000 f32 per bulk-copy chunk
IDXC = K // 16             # idx columns per window (16)
PAYC = (K // 128) * ES     # payload columns per window (128)

_NC_INPLACE = {}           # reps -> compiled Bacc program (no bulk copy)
_NC_COPY = {}              # reps -> compiled Bacc program (copy + scatter)
_INPLACE_OK = None         # tri-state: None = untested, True/False


def _build_inplace(reps=1):
    """Scatter-only program: output DRAM tensor is expected to alias the
    input logits buffer, so no bulk copy is emitted."""
    nc = bacc.Bacc("TRN2", target_bir_lowering=False,
                   dynamic_dma_scratch_size=65536)
    x = nc.dram_tensor("x", [N], mybir.dt.float32, kind="ExternalInput")
    pay = nc.dram_tensor("pay", [128, NWIN * PAYC], mybir.dt.float32,
                         kind="ExternalInput")
    idx = nc.dram_tensor("idx", [128, NWIN * IDXC], mybir.dt.int16,
                         kind="ExternalInput")
    out = nc.dram_tensor("out", [N], mybir.dt.float32, kind="ExternalOutput")

    with (
        nc.sbuf_tensor("pay_sb", [128, NWIN * PAYC], mybir.dt.float32) as pay_sb,
        nc.sbuf_tensor("idx_sb", [128, NWIN * IDXC], mybir.dt.int16) as idx_sb,
        nc.sbuf_tensor("touch_sb", [1, 64], mybir.dt.float32) as touch_sb,
        nc.semaphore("meta_sem") as meta_sem,
        nc.semaphore("pay_sem") as pay_sem,
        nc.semaphore("sc_sem") as sc_sem,
        nc.semaphore("prep_sem") as prep_sem,
    ):
        # Keep `x` referenced so its ExternalInput allocation survives
        # compilation (it is the alias target of `out`).
        nc.gpsimd.dma_start(
            touch_sb[:, :], x[0:64].rearrange("(a b) -> a b", a=1)
        ).then_inc(meta_sem, 16)
        for r in range(reps):
            # Fresh metadata load every rep so one rep == one full kernel
            # invocation.  The 262 KB payload rides a parallel HWDGE queue
            # (sync engine) while GPSIMD loads the 16 KB index array and
            # starts descriptor generation; only the doorbell needs the
            # payload to have landed.  Cross-rep: SBUF metadata may not be
            # overwritten until the previous rep's CCE transfers finished.
            if r > 0:
                nc.sync.wait_ge(sc_sem, 16 * NWIN * r)
            nc.sync.dma_start(pay_sb[:, :], pay[:, :]).then_inc(pay_sem, 16)
            nc.gpsimd.dma_start(idx_sb[:, :], idx[:, :]).then_inc(meta_sem, 16)
            nc.gpsimd.wait_ge(meta_sem, 16 + 16 * (r + 1))
            for w in range(NWIN):
                out_win = out[w * CHUNK:(w + 1) * CHUNK].rearrange(
                    "(a b) -> a b", b=ES)                       # [32000, 64]
                pay_ap = pay_sb[:, w * PAYC:(w + 1) * PAYC].rearrange(
                    "p (g e) -> p g e", e=ES)                   # [128, 2, 64]
                idx_ap = idx_sb[:, w * IDXC:(w + 1) * IDXC]     # [128, 16]
                nc.gpsimd.dma_scatter_add(
                    out_win, pay_ap, idx_ap, K, K, ES,
                    prepare_only=True, sem=sc_sem,
                ).then_inc(prep_sem, 1)
                # Fire this window as soon as its descriptors are committed:
                # its CCE transfer overlaps the next window's desc gen.
                nc.gpsimd.wait_ge(prep_sem, NWIN * r + w + 1)
                if w == 0:
                    nc.gpsimd.wait_ge(pay_sem, 16 * (r + 1))
                nc.gpsimd.trigger_dma(count=1)
            # All 4 windows' CCE transfers done before the next rep touches
            # the same addresses / SBUF metadata.
            nc.gpsimd.wait_ge(sc_sem, 16 * NWIN * (r + 1))
    nc.compile()
    return nc


def _build_copy(reps=1):
    """Previous proven kernel: bulk DRAM->DRAM copy + overlapped scatters."""
    nc = bacc.Bacc("TRN2", target_bir_lowering=False,
                   dynamic_dma_scratch_size=65536)
    x = nc.dram_tensor("x", [N], mybir.dt.float32, kind="ExternalInput")
    pay = nc.dram_tensor("pay", [128, NWIN * PAYC], mybir.dt.float32,
                         kind="ExternalInput")
    idx = nc.dram_tensor("idx", [128, NWIN * IDXC], mybir.dt.int16,
                         kind="ExternalInput")
    out = nc.dram_tensor("out", [N], mybir.dt.float32, kind="ExternalOutput")

    with (
        nc.sbuf_tensor("pay_sb", [128, NWIN * PAYC], mybir.dt.float32) as pay_sb,
        nc.sbuf_tensor("idx_sb", [128, NWIN * IDXC], mybir.dt.int16) as idx_sb,
        nc.semaphore("meta_sem") as meta_sem,
        nc.semaphore("copy_sem0") as cs0,
        nc.semaphore("copy_sem1") as cs1,
        nc.semaphore("copy_sem2") as cs2,
        nc.semaphore("copy_sem3") as cs3,
        nc.semaphore("sc_sem") as sc_sem,
        nc.semaphore("prep_sem") as prep_sem,
    ):
        copy_sems = [cs0, cs1, cs2, cs3]
        nc.gpsimd.dma_start(pay_sb[:, :], pay[:, :]).then_inc(meta_sem, 16)
        nc.gpsimd.dma_start(idx_sb[:, :], idx[:, :]).then_inc(meta_sem, 16)
        nc.gpsimd.wait_ge(meta_sem, 32)
        for r in range(reps):
            for w in range(NWIN):
                src = x[w * CHUNK:(w + 1) * CHUNK].rearrange(
                    "(a b) -> a b", b=16384)
                dst = out[w * CHUNK:(w + 1) * CHUNK].rearrange(
                    "(a b) -> a b", b=16384)
                eng = nc.sync if w % 2 == 0 else nc.scalar
                if r > 0:
                    # The previous rep's scatter on this chunk must land
                    # before we overwrite it with the fresh copy.
                    eng.wait_ge(sc_sem, 16 * NWIN * r)
                eng.dma_start(dst, src).then_inc(copy_sems[w], 16)
            for w in range(NWIN):
                out_win = out[w * CHUNK:(w + 1) * CHUNK].rearrange(
                    "(a b) -> a b", b=ES)                       # [32000, 64]
                pay_ap = pay_sb[:, w * PAYC:(w + 1) * PAYC].rearrange(
                    "p (g e) -> p g e", e=ES)                   # [128, 2, 64]
                idx_ap = idx_sb[:, w * IDXC:(w + 1) * IDXC]     # [128, 16]
                nc.gpsimd.dma_scatter_add(
                    out_win, pay_ap, idx_ap, K, K, ES,
                    prepare_only=True, sem=sc_sem,
                ).then_inc(prep_sem, 1)
            for w in range(NWIN):
                nc.gpsimd.wait_ge(prep_sem, NWIN * r + w + 1)
                nc.gpsimd.wait_ge(copy_sems[w], 16 * (r + 1))
                nc.gpsimd.trigger_dma(count=1)
        nc.gpsimd.wait_ge(sc_sem, 16 * NWIN * reps)
    nc.compile()
    return nc


def _get_nc_inplace(reps=1):
    if reps not in _NC_INPLACE:
        _NC_INPLACE[reps] = _build_inplace(reps)
    return _NC_INPLACE[reps]


def _get_nc_copy(reps=1):
    if reps not in _NC_COPY:
        _NC_COPY[reps] = _build_copy(reps)
    return _NC_COPY[reps]


# Back-compat alias for older harness code.
def _get_nc():
    return _get_nc_copy(1)


def _preprocess(tokens):
    """tokens [B, S] -> per-core scatter payload/index arrays.

    Returns (pay [8, 128, 512] f32, idx [8, 128, 64] int16).
    Slot k of window w holds one 64-float penalty vector targeting block
    idx[k%16, w*16 + k//16] (replicated across the 8 16-partition groups);
    its payload lives at pay[k%128, w*128 + (k//128)*64 : +64].
    All 4 slots of a row target distinct blocks within that row (padding
    slots point at untouched blocks with zero payload), so block ids within
    a scatter window are globally unique -> no RMW races.
    """
    tokens = np.asarray(tokens).astype(np.int64)
    pay_all = np.zeros((NCORES, 128, NWIN * PAYC), np.float32)
    idx_all = np.zeros((NCORES, 128, NWIN * IDXC), np.int16)
    for c in range(NCORES):
        pay, idx = pay_all[c], idx_all[c]
        for r in range(R):
            g = c * R + r
            b, i = divmod(g, S)
            w, rw = divmod(r, WROWS)
            upd = {}
            if i >= M:
                cols, cnts = np.unique(tokens[b, i - M:i], return_counts=True)
                for col, n in zip(cols, cnts):
                    cb, off = divmod(int(col), ES)
                    vec = upd.setdefault(cb, np.zeros(ES, np.float32))
                    vec[off] = -(W * np.float32(n))
            used = set(upd)
            entries = sorted(upd.items())
            t = 0
            while len(entries) < M:
                if t not in used:
                    entries.append((t, None))
                t += 1
            for j, (cb, vec) in enumerate(entries):
                k = rw * M + j
                idx[k % 16::16, w * IDXC + k // 16] = rw * BPR + cb
                if vec is not None:
                    base = w * PAYC + (k // 128) * ES
                    pay[k % 128, base:base + ES] = vec
    return pay_all, idx_all


_EXEC_CACHE = {}


def _get_exec(nc, alias_out_to=None, donate_inputs=()):
    """Build (once) and cache the sharded PJRT executable for a compiled
    Bacc program.  `alias_out_to`: {output_name: input_name} XLA aliases.
    Returns (sharded_fn, in_names, out_names, out_avals, sharding)."""
    import jax
    from jax.sharding import Mesh, PartitionSpec, NamedSharding
    from jax.experimental.shard_map import shard_map
    import concourse.bass2jax as b2j

    key = (id(nc), tuple(sorted((alias_out_to or {}).items())),
           tuple(donate_inputs))
    if key in _EXEC_CACHE:
        return _EXEC_CACHE[key]

    b2j.install_neuronx_cc_hook()
    partition_name = (nc.partition_id_tensor.name
                      if nc.partition_id_tensor else None)
    in_names, out_names, out_avals = [], [], []
    for alloc in nc.m.functions[0].allocations:
        if not isinstance(alloc, mybir.MemoryLocationSet):
            continue
        name = alloc.memorylocations[0].name
        if alloc.kind == "ExternalInput":
            if name != partition_name:
                in_names.append(name)
        elif alloc.kind == "ExternalOutput":
            out_names.append(name)
            shape = tuple(alloc.tensor_shape)
            dtype = mybir.dt.np(alloc.dtype)
            out_avals.append(jax.core.ShapedArray(shape, dtype))
    aliases = tuple(
        (out_names.index(o), in_names.index(i))
        for o, i in (alias_out_to or {}).items()
    )
    donate = tuple(in_names.index(i) for i in donate_inputs)
    all_in_names = list(in_names)
    if partition_name is not None:
        all_in_names.append(partition_name)

    def _body(*args):
        operands = list(args)
        if partition_name is not None:
            operands.append(b2j.partition_id_tensor())
        outs = b2j._bass_exec_p.bind(
            *operands,
            out_avals=tuple(out_avals),
            in_names=tuple(all_in_names),
            out_names=tuple(out_names),
            lowering_input_output_aliases=aliases,
            sim_require_finite=True,
            sim_require_nnan=True,
            nc=nc,
        )
        return tuple(outs)

    devices = jax.devices()[:NCORES]
    mesh = Mesh(np.asarray(devices), ("core",))
    spec = PartitionSpec("core")
    sharded = jax.jit(
        shard_map(_body, mesh=mesh,
                  in_specs=(spec,) * len(in_names),
                  out_specs=(spec,) * len(out_names),
                  check_rep=False),
        donate_argnums=donate,
        keep_unused=True,
    )
    sharding = NamedSharding(mesh, spec)
    entry = (sharded, in_names, out_names, out_avals, sharding)
    _EXEC_CACHE[key] = entry
    return entry


def _exec_spmd(nc, in_maps, alias_out_to=None, donate_inputs=()):
    """Run a compiled Bacc program on NCORES cores via the bass2jax PJRT
    path.  Returns list (per core) of {output_name: np.ndarray}."""
    import jax

    sharded, in_names, out_names, out_avals, sharding = _get_exec(
        nc, alias_out_to, donate_inputs)
    concat_in = [
        jax.device_put(
            np.concatenate([np.asarray(in_maps[c][nm]) for c in range(NCORES)],
                           axis=0), sharding)
        for nm in in_names
    ]
    out_arrs = sharded(*concat_in)
    jax.block_until_ready(out_arrs)
    return [
        {
            name: np.asarray(out_arrs[i]).reshape(NCORES, *out_avals[i].shape)[c]
            for i, name in enumerate(out_names)
        }
        for c in range(NCORES)
    ]


def _expected_updates(logits_flat, tokens):
    """Exact expected values at every penalized element: list of
    (row, col, expected_value) computed on host from the tiny token tensor."""
    tokens = np.asarray(tokens).astype(np.int64)
    updates = []
    for g in range(B * S):
        b, i = divmod(g, S)
        if i < M:
            continue
        cols, cnts = np.unique(tokens[b, i - M:i], return_counts=True)
        for col, n in zip(cols, cnts):
            updates.append((g, int(col),
                            logits_flat[g, int(col)]
                            - np.float32(0.3) * np.float32(n)))
    return updates


def kernel(logits, generated_tokens):
    global _INPLACE_OK
    logits = np.ascontiguousarray(np.asarray(logits, dtype=np.float32))
    pay_all, idx_all = _preprocess(generated_tokens)
    flat = logits.reshape(B * S, V)
    in_maps = [
        {
            "x": np.ascontiguousarray(flat[c * R:(c + 1) * R]).reshape(N),
            "pay": pay_all[c],
            "idx": idx_all[c],
        }
        for c in range(NCORES)
    ]

    if _INPLACE_OK is not False:
        res = _exec_spmd(_get_nc_inplace(1), in_maps,
                         alias_out_to={"out": "x"}, donate_inputs=("x",))
        out = np.concatenate([res[c]["out"] for c in range(NCORES)])
        out = out.reshape(B * S, V)
        if _INPLACE_OK is None:
            # Validate the aliasing path once: every penalized element must
            # match the host-exact value, and untouched elements must equal
            # the input.
            ok = True
            for g, col, exp in _expected_updates(flat, generated_tokens):
                if out[g, col] != exp:
                    ok = False
                    break
            if ok:
                rng = np.random.default_rng(0)
                gs = rng.integers(0, B * S, 2048)
                cs = rng.integers(0, V, 2048)
                touched = {(g, c) for g, c, _ in
                           _expected_updates(flat, generated_tokens)}
                for g, c in zip(gs, cs):
                    if (int(g), int(c)) in touched:
                        continue
                    if out[int(g), int(c)] != flat[int(g), int(c)]:
                        ok = False
                        break
            _INPLACE_OK = bool(ok)
            if not ok:
                # Aliasing not honored in this environment; fall back.
                return kernel(logits, generated_tokens)
        return out.reshape(B, S, V)

    from concourse.bass_utils import run_bass_kernel_spmd
    res = run_bass_kernel_spmd(_get_nc_copy(1), in_maps,
                               core_ids=list(range(NCORES)))
    out = np.concatenate([res.results[c]["out"] for c in range(NCORES)])
    return out.reshape(B, S, V)


# revision 9
# speedup vs baseline: 1.2838x; 1.2838x over previous
"""Trainium2 Bass kernel for the CoverageMechanism (repeat-penalty) problem.

Reference semantics: for logits [B=4, S=512, V=32000] and generated_tokens
[B, S], the output is

    out[b, i, v] = logits[b, i, v] - 0.3 * #{j in [i-4, i) : tokens[b, j] == v}

for i >= 4, and out = logits for i < 4.  That is a 262 MB identity map plus
an extremely sparse update: each (b, i) row of 32000 floats gets at most 4
elements decremented.

Strategy (8 NeuronCores):
  - Flatten (b, i) to 2048 rows, shard 256 rows per core (window never
    crosses a batch row boundary; host has all tokens so no halo exchange).
  - Host preprocesses the 8 KB token tensor into per-core DMA scatter-add
    metadata (block indices + 64-float payload vectors holding -0.3*count).
  - Device (primary, in-place): the kernel output DRAM tensor is aliased to
    the donated logits input buffer (XLA input/output aliasing through
    bass2jax's `lowering_input_output_aliases`), so the 32.75 MB per-core
    bulk copy disappears entirely.  The kernel is then just: load ~280 KB of
    scatter metadata to SBUF, generate the SWDGE descriptors (GPSIMD Q7),
    and fire 4 dma_scatter_add calls (CCE read-modify-write add) that apply
    the sparse penalties in place on the logits buffer.  Each scatter covers
    64 rows (32000 blocks of 64 floats -> block ids fit int16) and all 256
    block indices within a scatter are unique, so the CCE RMW has no
    same-address races.
  - kernel() verifies the in-place result against a host-computed exact
    check on every updated element (plus untouched samples); if the aliasing
    path is ever unavailable it falls back to the previous proven
    copy-then-scatter kernel (DRAM->DRAM copy overlapped with scatters).

Both program builders accept `reps`: the same kernel body unrolled N times
with full cross-rep synchronization.  A fresh-metadata load is included in
every rep, so one rep == one complete kernel invocation; harnesses use the
wall-time slope over `reps` to measure per-invocation device time without
the multi-ms host->device dispatch overhead of the axon relay.
"""

import numpy as np

import concourse.bass as bass  # noqa: F401  (kept for parity with bacc deps)
import concourse.bacc as bacc
import concourse.mybir as mybir

B, S, V = 4, 512, 32000
M = 4                      # sliding window length
W = np.float32(0.3)        # penalty weight
NCORES = 8
R = (B * S) // NCORES      # 256 rows per core
N = R * V                  # 8_192_000 f32 per core
NWIN = 4                   # scatter windows per core
WROWS = R // NWIN          # 64 rows per window
K = WROWS * M              # 256 scatter slots per window
ES = 64                    # scatter elem_size (64 f32 = 256 B blocks)
BPR = V // ES              # 500 blocks per row
CHUNK = N // NWIN          # 2_048_000 f32 per bulk-copy chunk
IDXC = K // 16             # idx columns per window (16)
PAYC = (K // 128) * ES     # payload columns per window (128)

_NC_INPLACE = {}           # reps -> compiled Bacc program (no bulk copy)
_NC_COPY = {}              # reps -> compiled Bacc program (copy + scatter)
_INPLACE_OK = None         # tri-state: None = untested, True/False


def _build_inplace(reps=1):
    """Scatter-only program: output DRAM tensor is expected to alias the
    input logits buffer, so no bulk copy is emitted."""
    nc = bacc.Bacc("TRN2", target_bir_lowering=False,
                   dynamic_dma_scratch_size=65536)
    x = nc.dram_tensor("x", [N], mybir.dt.float32, kind="ExternalInput")
    pay = nc.dram_tensor("pay", [128, NWIN * PAYC], mybir.dt.float32,
                         kind="ExternalInput")
    idx = nc.dram_tensor("idx", [128, NWIN * IDXC], mybir.dt.int16,
                         kind="ExternalInput")
    out = nc.dram_tensor("out", [N], mybir.dt.float32, kind="ExternalOutput")

    with (
        nc.sbuf_tensor("pay_sb", [128, NWIN * PAYC], mybir.dt.float32) as pay_sb,
        nc.sbuf_tensor("idx_sb", [128, NWIN * IDXC], mybir.dt.int16) as idx_sb,
        nc.sbuf_tensor("touch_sb", [1, 64], mybir.dt.float32) as touch_sb,
        nc.semaphore("meta_sem") as meta_sem,
        nc.semaphore("sc_sem") as sc_sem,
        nc.semaphore("prep_sem") as prep_sem,
    ):
        # Keep `x` referenced so its ExternalInput allocation survives
        # compilation (it is the alias target of `out`).
        nc.gpsimd.dma_start(
            touch_sb[:, :], x[0:64].rearrange("(a b) -> a b", a=1)
        ).then_inc(meta_sem, 16)
        for r in range(reps):
            # Fresh metadata load every rep so one rep == one full kernel
            # invocation.  GPSIMD program order plus the previous rep's
            # trailing sc_sem wait guarantees the SBUF metadata is not
            # overwritten while the previous rep's CCE transfers still read
            # it.  (A/B-measured: splitting the payload load onto a parallel
            # HWDGE queue or firing each window's doorbell right after its
            # prep is within noise or slightly slower than this simple
            # serial structure; descriptor generation ~7 us dominates.)
            nc.gpsimd.dma_start(pay_sb[:, :], pay[:, :]).then_inc(meta_sem, 16)
            nc.gpsimd.dma_start(idx_sb[:, :], idx[:, :]).then_inc(meta_sem, 16)
            nc.gpsimd.wait_ge(meta_sem, 16 + 32 * (r + 1))
            for w in range(NWIN):
                out_win = out[w * CHUNK:(w + 1) * CHUNK].rearrange(
                    "(a b) -> a b", b=ES)                       # [32000, 64]
                pay_ap = pay_sb[:, w * PAYC:(w + 1) * PAYC].rearrange(
                    "p (g e) -> p g e", e=ES)                   # [128, 2, 64]
                idx_ap = idx_sb[:, w * IDXC:(w + 1) * IDXC]     # [128, 16]
                nc.gpsimd.dma_scatter_add(
                    out_win, pay_ap, idx_ap, K, K, ES,
                    prepare_only=True, sem=sc_sem,
                ).then_inc(prep_sem, 1)
            for w in range(NWIN):
                nc.gpsimd.wait_ge(prep_sem, NWIN * r + w + 1)
                nc.gpsimd.trigger_dma(count=1)
            # All 4 windows' CCE transfers done before the next rep touches
            # the same addresses / SBUF metadata.
            nc.gpsimd.wait_ge(sc_sem, 16 * NWIN * (r + 1))
    nc.compile()
    return nc


def _build_copy(reps=1):
    """Previous proven kernel: bulk DRAM->DRAM copy + overlapped scatters."""
    nc = bacc.Bacc("TRN2", target_bir_lowering=False,
                   dynamic_dma_scratch_size=65536)
    x = nc.dram_tensor("x", [N], mybir.dt.float32, kind="ExternalInput")
    pay = nc.dram_tensor("pay", [128, NWIN * PAYC], mybir.dt.float32,
                         kind="ExternalInput")
    idx = nc.dram_tensor("idx", [128, NWIN * IDXC], mybir.dt.int16,
                         kind="ExternalInput")
    out = nc.dram_tensor("out", [N], mybir.dt.float32, kind="ExternalOutput")

    with (
        nc.sbuf_tensor("pay_sb", [128, NWIN * PAYC], mybir.dt.float32) as pay_sb,
        nc.sbuf_tensor("idx_sb", [128, NWIN * IDXC], mybir.dt.int16) as idx_sb,
        nc.semaphore("meta_sem") as meta_sem,
        nc.semaphore("copy_sem0") as cs0,
        nc.semaphore("copy_sem1") as cs1,
        nc.semaphore("copy_sem2") as cs2,
        nc.semaphore("copy_sem3") as cs3,
        nc.semaphore("sc_sem") as sc_sem,
        nc.semaphore("prep_sem") as prep_sem,
    ):
        copy_sems = [cs0, cs1, cs2, cs3]
        nc.gpsimd.dma_start(pay_sb[:, :], pay[:, :]).then_inc(meta_sem, 16)
        nc.gpsimd.dma_start(idx_sb[:, :], idx[:, :]).then_inc(meta_sem, 16)
        nc.gpsimd.wait_ge(meta_sem, 32)
        for r in range(reps):
            for w in range(NWIN):
                src = x[w * CHUNK:(w + 1) * CHUNK].rearrange(
                    "(a b) -> a b", b=16384)
                dst = out[w * CHUNK:(w + 1) * CHUNK].rearrange(
                    "(a b) -> a b", b=16384)
                eng = nc.sync if w % 2 == 0 else nc.scalar
                if r > 0:
                    # The previous rep's scatter on this chunk must land
                    # before we overwrite it with the fresh copy.
                    eng.wait_ge(sc_sem, 16 * NWIN * r)
                eng.dma_start(dst, src).then_inc(copy_sems[w], 16)
            for w in range(NWIN):
                out_win = out[w * CHUNK:(w + 1) * CHUNK].rearrange(
                    "(a b) -> a b", b=ES)                       # [32000, 64]
                pay_ap = pay_sb[:, w * PAYC:(w + 1) * PAYC].rearrange(
                    "p (g e) -> p g e", e=ES)                   # [128, 2, 64]
                idx_ap = idx_sb[:, w * IDXC:(w + 1) * IDXC]     # [128, 16]
                nc.gpsimd.dma_scatter_add(
                    out_win, pay_ap, idx_ap, K, K, ES,
                    prepare_only=True, sem=sc_sem,
                ).then_inc(prep_sem, 1)
            for w in range(NWIN):
                nc.gpsimd.wait_ge(prep_sem, NWIN * r + w + 1)
                nc.gpsimd.wait_ge(copy_sems[w], 16 * (r + 1))
                nc.gpsimd.trigger_dma(count=1)
        nc.gpsimd.wait_ge(sc_sem, 16 * NWIN * reps)
    nc.compile()
    return nc


def _get_nc_inplace(reps=1):
    if reps not in _NC_INPLACE:
        _NC_INPLACE[reps] = _build_inplace(reps)
    return _NC_INPLACE[reps]


def _get_nc_copy(reps=1):
    if reps not in _NC_COPY:
        _NC_COPY[reps] = _build_copy(reps)
    return _NC_COPY[reps]


# Back-compat alias for older harness code.
def _get_nc():
    return _get_nc_copy(1)


def _preprocess(tokens):
    """tokens [B, S] -> per-core scatter payload/index arrays.

    Returns (pay [8, 128, 512] f32, idx [8, 128, 64] int16).
    Slot k of window w holds one 64-float penalty vector targeting block
    idx[k%16, w*16 + k//16] (replicated across the 8 16-partition groups);
    its payload lives at pay[k%128, w*128 + (k//128)*64 : +64].
    All 4 slots of a row target distinct blocks within that row (padding
    slots point at untouched blocks with zero payload), so block ids within
    a scatter window are globally unique -> no RMW races.
    """
    tokens = np.asarray(tokens).astype(np.int64)
    pay_all = np.zeros((NCORES, 128, NWIN * PAYC), np.float32)
    idx_all = np.zeros((NCORES, 128, NWIN * IDXC), np.int16)
    for c in range(NCORES):
        pay, idx = pay_all[c], idx_all[c]
        for r in range(R):
            g = c * R + r
            b, i = divmod(g, S)
            w, rw = divmod(r, WROWS)
            upd = {}
            if i >= M:
                cols, cnts = np.unique(tokens[b, i - M:i], return_counts=True)
                for col, n in zip(cols, cnts):
                    cb, off = divmod(int(col), ES)
                    vec = upd.setdefault(cb, np.zeros(ES, np.float32))
                    vec[off] = -(W * np.float32(n))
            used = set(upd)
            entries = sorted(upd.items())
            t = 0
            while len(entries) < M:
                if t not in used:
                    entries.append((t, None))
                t += 1
            for j, (cb, vec) in enumerate(entries):
                k = rw * M + j
                idx[k % 16::16, w * IDXC + k // 16] = rw * BPR + cb
                if vec is not None:
                    base = w * PAYC + (k // 128) * ES
                    pay[k % 128, base:base + ES] = vec
    return pay_all, idx_all


_EXEC_CACHE = {}


def _get_exec(nc, alias_out_to=None, donate_inputs=()):
    """Build (once) and cache the sharded PJRT executable for a compiled
    Bacc program.  `alias_out_to`: {output_name: input_name} XLA aliases.
    Returns (sharded_fn, in_names, out_names, out_avals, sharding)."""
    import jax
    from jax.sharding import Mesh, PartitionSpec, NamedSharding
    from jax.experimental.shard_map import shard_map
    import concourse.bass2jax as b2j

    key = (id(nc), tuple(sorted((alias_out_to or {}).items())),
           tuple(donate_inputs))
    if key in _EXEC_CACHE:
        return _EXEC_CACHE[key]

    b2j.install_neuronx_cc_hook()
    partition_name = (nc.partition_id_tensor.name
                      if nc.partition_id_tensor else None)
    in_names, out_names, out_avals = [], [], []
    for alloc in nc.m.functions[0].allocations:
        if not isinstance(alloc, mybir.MemoryLocationSet):
            continue
        name = alloc.memorylocations[0].name
        if alloc.kind == "ExternalInput":
            if name != partition_name:
                in_names.append(name)
        elif alloc.kind == "ExternalOutput":
            out_names.append(name)
            shape = tuple(alloc.tensor_shape)
            dtype = mybir.dt.np(alloc.dtype)
            out_avals.append(jax.core.ShapedArray(shape, dtype))
    aliases = tuple(
        (out_names.index(o), in_names.index(i))
        for o, i in (alias_out_to or {}).items()
    )
    donate = tuple(in_names.index(i) for i in donate_inputs)
    all_in_names = list(in_names)
    if partition_name is not None:
        all_in_names.append(partition_name)

    def _body(*args):
        operands = list(args)
        if partition_name is not None:
            operands.append(b2j.partition_id_tensor())
        outs = b2j._bass_exec_p.bind(
            *operands,
            out_avals=tuple(out_avals),
            in_names=tuple(all_in_names),
            out_names=tuple(out_names),
            lowering_input_output_aliases=aliases,
            sim_require_finite=True,
            sim_require_nnan=True,
            nc=nc,
        )
        return tuple(outs)

    devices = jax.devices()[:NCORES]
    mesh = Mesh(np.asarray(devices), ("core",))
    spec = PartitionSpec("core")
    sharded = jax.jit(
        shard_map(_body, mesh=mesh,
                  in_specs=(spec,) * len(in_names),
                  out_specs=(spec,) * len(out_names),
                  check_rep=False),
        donate_argnums=donate,
        keep_unused=True,
    )
    sharding = NamedSharding(mesh, spec)
    entry = (sharded, in_names, out_names, out_avals, sharding)
    _EXEC_CACHE[key] = entry
    return entry


def _exec_spmd(nc, in_maps, alias_out_to=None, donate_inputs=()):
    """Run a compiled Bacc program on NCORES cores via the bass2jax PJRT
    path.  Returns list (per core) of {output_name: np.ndarray}."""
    import jax

    sharded, in_names, out_names, out_avals, sharding = _get_exec(
        nc, alias_out_to, donate_inputs)
    concat_in = [
        jax.device_put(
            np.concatenate([np.asarray(in_maps[c][nm]) for c in range(NCORES)],
                           axis=0), sharding)
        for nm in in_names
    ]
    out_arrs = sharded(*concat_in)
    jax.block_until_ready(out_arrs)
    return [
        {
            name: np.asarray(out_arrs[i]).reshape(NCORES, *out_avals[i].shape)[c]
            for i, name in enumerate(out_names)
        }
        for c in range(NCORES)
    ]


def _expected_updates(logits_flat, tokens):
    """Exact expected values at every penalized element: list of
    (row, col, expected_value) computed on host from the tiny token tensor."""
    tokens = np.asarray(tokens).astype(np.int64)
    updates = []
    for g in range(B * S):
        b, i = divmod(g, S)
        if i < M:
            continue
        cols, cnts = np.unique(tokens[b, i - M:i], return_counts=True)
        for col, n in zip(cols, cnts):
            updates.append((g, int(col),
                            logits_flat[g, int(col)]
                            - np.float32(0.3) * np.float32(n)))
    return updates


def kernel(logits, generated_tokens):
    global _INPLACE_OK
    logits = np.ascontiguousarray(np.asarray(logits, dtype=np.float32))
    pay_all, idx_all = _preprocess(generated_tokens)
    flat = logits.reshape(B * S, V)
    in_maps = [
        {
            "x": np.ascontiguousarray(flat[c * R:(c + 1) * R]).reshape(N),
            "pay": pay_all[c],
            "idx": idx_all[c],
        }
        for c in range(NCORES)
    ]

    if _INPLACE_OK is not False:
        res = _exec_spmd(_get_nc_inplace(1), in_maps,
                         alias_out_to={"out": "x"}, donate_inputs=("x",))
        out = np.concatenate([res[c]["out"] for c in range(NCORES)])
        out = out.reshape(B * S, V)
        if _INPLACE_OK is None:
            # Validate the aliasing path once: every penalized element must
            # match the host-exact value, and untouched elements must equal
            # the input.
            ok = True
            for g, col, exp in _expected_updates(flat, generated_tokens):
                if out[g, col] != exp:
                    ok = False
                    break
            if ok:
                rng = np.random.default_rng(0)
                gs = rng.integers(0, B * S, 2048)
                cs = rng.integers(0, V, 2048)
                touched = {(g, c) for g, c, _ in
                           _expected_updates(flat, generated_tokens)}
                for g, c in zip(gs, cs):
                    if (int(g), int(c)) in touched:
                        continue
                    if out[int(g), int(c)] != flat[int(g), int(c)]:
                        ok = False
                        break
            _INPLACE_OK = bool(ok)
            if not ok:
                # Aliasing not honored in this environment; fall back.
                return kernel(logits, generated_tokens)
        return out.reshape(B, S, V)

    from concourse.bass_utils import run_bass_kernel_spmd
    res = run_bass_kernel_spmd(_get_nc_copy(1), in_maps,
                               core_ids=list(range(NCORES)))
    out = np.concatenate([res.results[c]["out"] for c in range(NCORES)])
    return out.reshape(B, S, V)


# revision 11
# speedup vs baseline: 1.4662x; 1.1421x over previous
"""Trainium2 Bass kernel for the CoverageMechanism (repeat-penalty) problem.

Reference semantics: for logits [B=4, S=512, V=32000] and generated_tokens
[B, S], the output is

    out[b, i, v] = logits[b, i, v] - 0.3 * #{j in [i-4, i) : tokens[b, j] == v}

for i >= 4, and out = logits for i < 4.  That is a 262 MB identity map plus
an extremely sparse update: each (b, i) row of 32000 floats gets at most 4
elements decremented.

Strategy (8 NeuronCores):
  - Flatten (b, i) to 2048 rows, shard 256 rows per core (window never
    crosses a batch row boundary; host has all tokens so no halo exchange).
  - Host preprocesses the 8 KB token tensor into per-core DMA scatter-add
    metadata (block indices + 64-float payload vectors holding -0.3*count).
  - Device (primary, in-place): the kernel output DRAM tensor is aliased to
    the donated logits input buffer (XLA input/output aliasing through
    bass2jax's `lowering_input_output_aliases`), so the 32.75 MB per-core
    bulk copy disappears entirely.  The kernel is then just: load ~280 KB of
    scatter metadata to SBUF, generate the SWDGE descriptors (GPSIMD Q7),
    and fire 4 dma_scatter_add calls (CCE read-modify-write add) that apply
    the sparse penalties in place on the logits buffer.  Each scatter covers
    64 rows (32000 blocks of 64 floats -> block ids fit int16) and all 256
    block indices within a scatter are unique, so the CCE RMW has no
    same-address races.
  - kernel() verifies the in-place result against a host-computed exact
    check on every updated element (plus untouched samples); if the aliasing
    path is ever unavailable it falls back to the previous proven
    copy-then-scatter kernel (DRAM->DRAM copy overlapped with scatters).

Both program builders accept `reps`: the same kernel body unrolled N times
with full cross-rep synchronization.  A fresh-metadata load is included in
every rep, so one rep == one complete kernel invocation; harnesses use the
wall-time slope over `reps` to measure per-invocation device time without
the multi-ms host->device dispatch overhead of the axon relay.
"""

import numpy as np

import concourse.bass as bass  # noqa: F401  (kept for parity with bacc deps)
import concourse.bacc as bacc
import concourse.mybir as mybir

B, S, V = 4, 512, 32000
M = 4                      # sliding window length
W = np.float32(0.3)        # penalty weight
NCORES = 8
R = (B * S) // NCORES      # 256 rows per core
N = R * V                  # 8_192_000 f32 per core
NWIN = 4                   # scatter windows per core
WROWS = R // NWIN          # 64 rows per window
K = WROWS * M              # 256 scatter slots per window
ES = 64                    # scatter elem_size (64 f32 = 256 B blocks)
BPR = V // ES              # 500 blocks per row
CHUNK = N // NWIN          # 2_048_000 f32 per bulk-copy chunk
IDXC = K // 16             # idx columns per window (16)
PAYC = (K // 128) * ES     # payload columns per window (128)

_NC_INPLACE = {}           # reps -> compiled Bacc program (no bulk copy)
_NC_COPY = {}              # reps -> compiled Bacc program (copy + scatter)
_INPLACE_OK = None         # tri-state: None = untested, True/False


def _build_inplace(reps=1):
    """Scatter-only program: output DRAM tensor is expected to alias the
    input logits buffer, so no bulk copy is emitted."""
    nc = bacc.Bacc("TRN2", target_bir_lowering=False,
                   dynamic_dma_scratch_size=65536)
    x = nc.dram_tensor("x", [N], mybir.dt.float32, kind="ExternalInput")
    pay = nc.dram_tensor("pay", [128, NWIN * PAYC], mybir.dt.float32,
                         kind="ExternalInput")
    idx = nc.dram_tensor("idx", [128, NWIN * IDXC], mybir.dt.int16,
                         kind="ExternalInput")
    out = nc.dram_tensor("out", [N], mybir.dt.float32, kind="ExternalOutput")

    with (
        nc.sbuf_tensor("pay_sb", [128, NWIN * PAYC], mybir.dt.float32) as pay_sb,
        nc.sbuf_tensor("idx_sb", [128, NWIN * IDXC], mybir.dt.int16) as idx_sb,
        nc.sbuf_tensor("touch_sb", [1, 64], mybir.dt.float32) as touch_sb,
        nc.semaphore("meta_sem") as meta_sem,
        nc.semaphore("pay_sem") as pay_sem,
        nc.semaphore("sc_sem") as sc_sem,
        nc.semaphore("prep_sem") as prep_sem,
    ):
        # Keep `x` referenced so its ExternalInput allocation survives
        # compilation (it is the alias target of `out`).
        nc.gpsimd.dma_start(
            touch_sb[:, :], x[0:64].rearrange("(a b) -> a b", a=1)
        ).then_inc(meta_sem, 16)
        for r in range(reps):
            # Fresh metadata load every rep so one rep == one full kernel
            # invocation.  Descriptor generation only reads the 16 KB index
            # array, so it is loaded first and gen starts as soon as it
            # lands; the 262 KB payload transfer overlaps generation and is
            # only gated at the first doorbell (the CCE transfers read it).
            # GPSIMD program order plus the previous rep's trailing sc_sem
            # wait guarantees the SBUF metadata is not overwritten while the
            # previous rep's CCE transfers still read it.
            nc.gpsimd.dma_start(idx_sb[:, :], idx[:, :]).then_inc(meta_sem, 16)
            nc.gpsimd.dma_start(pay_sb[:, :], pay[:, :]).then_inc(pay_sem, 16)
            nc.gpsimd.wait_ge(meta_sem, 16 + 16 * (r + 1))
            for w in range(NWIN):
                out_win = out[w * CHUNK:(w + 1) * CHUNK].rearrange(
                    "(a b) -> a b", b=ES)                       # [32000, 64]
                pay_ap = pay_sb[:, w * PAYC:(w + 1) * PAYC].rearrange(
                    "p (g e) -> p g e", e=ES)                   # [128, 2, 64]
                idx_ap = idx_sb[:, w * IDXC:(w + 1) * IDXC]     # [128, 16]
                nc.gpsimd.dma_scatter_add(
                    out_win, pay_ap, idx_ap, K, K, ES,
                    prepare_only=True, sem=sc_sem,
                ).then_inc(prep_sem, 1)
            for w in range(NWIN):
                nc.gpsimd.wait_ge(prep_sem, NWIN * r + w + 1)
                if w == 0:
                    nc.gpsimd.wait_ge(pay_sem, 16 * (r + 1))
                nc.gpsimd.trigger_dma(count=1)
            # All 4 windows' CCE transfers done before the next rep touches
            # the same addresses / SBUF metadata.
            nc.gpsimd.wait_ge(sc_sem, 16 * NWIN * (r + 1))
    nc.compile()
    return nc


def _build_copy(reps=1):
    """Previous proven kernel: bulk DRAM->DRAM copy + overlapped scatters."""
    nc = bacc.Bacc("TRN2", target_bir_lowering=False,
                   dynamic_dma_scratch_size=65536)
    x = nc.dram_tensor("x", [N], mybir.dt.float32, kind="ExternalInput")
    pay = nc.dram_tensor("pay", [128, NWIN * PAYC], mybir.dt.float32,
                         kind="ExternalInput")
    idx = nc.dram_tensor("idx", [128, NWIN * IDXC], mybir.dt.int16,
                         kind="ExternalInput")
    out = nc.dram_tensor("out", [N], mybir.dt.float32, kind="ExternalOutput")

    with (
        nc.sbuf_tensor("pay_sb", [128, NWIN * PAYC], mybir.dt.float32) as pay_sb,
        nc.sbuf_tensor("idx_sb", [128, NWIN * IDXC], mybir.dt.int16) as idx_sb,
        nc.semaphore("meta_sem") as meta_sem,
        nc.semaphore("copy_sem0") as cs0,
        nc.semaphore("copy_sem1") as cs1,
        nc.semaphore("copy_sem2") as cs2,
        nc.semaphore("copy_sem3") as cs3,
        nc.semaphore("sc_sem") as sc_sem,
        nc.semaphore("prep_sem") as prep_sem,
    ):
        copy_sems = [cs0, cs1, cs2, cs3]
        nc.gpsimd.dma_start(pay_sb[:, :], pay[:, :]).then_inc(meta_sem, 16)
        nc.gpsimd.dma_start(idx_sb[:, :], idx[:, :]).then_inc(meta_sem, 16)
        nc.gpsimd.wait_ge(meta_sem, 32)
        for r in range(reps):
            for w in range(NWIN):
                src = x[w * CHUNK:(w + 1) * CHUNK].rearrange(
                    "(a b) -> a b", b=16384)
                dst = out[w * CHUNK:(w + 1) * CHUNK].rearrange(
                    "(a b) -> a b", b=16384)
                eng = nc.sync if w % 2 == 0 else nc.scalar
                if r > 0:
                    # The previous rep's scatter on this chunk must land
                    # before we overwrite it with the fresh copy.
                    eng.wait_ge(sc_sem, 16 * NWIN * r)
                eng.dma_start(dst, src).then_inc(copy_sems[w], 16)
            for w in range(NWIN):
                out_win = out[w * CHUNK:(w + 1) * CHUNK].rearrange(
                    "(a b) -> a b", b=ES)                       # [32000, 64]
                pay_ap = pay_sb[:, w * PAYC:(w + 1) * PAYC].rearrange(
                    "p (g e) -> p g e", e=ES)                   # [128, 2, 64]
                idx_ap = idx_sb[:, w * IDXC:(w + 1) * IDXC]     # [128, 16]
                nc.gpsimd.dma_scatter_add(
                    out_win, pay_ap, idx_ap, K, K, ES,
                    prepare_only=True, sem=sc_sem,
                ).then_inc(prep_sem, 1)
            for w in range(NWIN):
                nc.gpsimd.wait_ge(prep_sem, NWIN * r + w + 1)
                nc.gpsimd.wait_ge(copy_sems[w], 16 * (r + 1))
                nc.gpsimd.trigger_dma(count=1)
        nc.gpsimd.wait_ge(sc_sem, 16 * NWIN * reps)
    nc.compile()
    return nc


def _get_nc_inplace(reps=1):
    if reps not in _NC_INPLACE:
        _NC_INPLACE[reps] = _build_inplace(reps)
    return _NC_INPLACE[reps]


def _get_nc_copy(reps=1):
    if reps not in _NC_COPY:
        _NC_COPY[reps] = _build_copy(reps)
    return _NC_COPY[reps]


# Back-compat alias for older harness code.
def _get_nc():
    return _get_nc_copy(1)


def _preprocess(tokens):
    """tokens [B, S] -> per-core scatter payload/index arrays.

    Returns (pay [8, 128, 512] f32, idx [8, 128, 64] int16).
    Slot k of window w holds one 64-float penalty vector targeting block
    idx[k%16, w*16 + k//16] (replicated across the 8 16-partition groups);
    its payload lives at pay[k%128, w*128 + (k//128)*64 : +64].
    All 4 slots of a row target distinct blocks within that row (padding
    slots point at untouched blocks with zero payload), so block ids within
    a scatter window are globally unique -> no RMW races.
    """
    tokens = np.asarray(tokens).astype(np.int64)
    pay_all = np.zeros((NCORES, 128, NWIN * PAYC), np.float32)
    idx_all = np.zeros((NCORES, 128, NWIN * IDXC), np.int16)
    for c in range(NCORES):
        pay, idx = pay_all[c], idx_all[c]
        for r in range(R):
            g = c * R + r
            b, i = divmod(g, S)
            w, rw = divmod(r, WROWS)
            upd = {}
            if i >= M:
                cols, cnts = np.unique(tokens[b, i - M:i], return_counts=True)
                for col, n in zip(cols, cnts):
                    cb, off = divmod(int(col), ES)
                    vec = upd.setdefault(cb, np.zeros(ES, np.float32))
                    vec[off] = -(W * np.float32(n))
            used = set(upd)
            entries = sorted(upd.items())
            t = 0
            while len(entries) < M:
                if t not in used:
                    entries.append((t, None))
                t += 1
            for j, (cb, vec) in enumerate(entries):
                k = rw * M + j
                idx[k % 16::16, w * IDXC + k // 16] = rw * BPR + cb
                if vec is not None:
                    base = w * PAYC + (k // 128) * ES
                    pay[k % 128, base:base + ES] = vec
    return pay_all, idx_all


_EXEC_CACHE = {}


def _get_exec(nc, alias_out_to=None, donate_inputs=()):
    """Build (once) and cache the sharded PJRT executable for a compiled
    Bacc program.  `alias_out_to`: {output_name: input_name} XLA aliases.
    Returns (sharded_fn, in_names, out_names, out_avals, sharding)."""
    import jax
    from jax.sharding import Mesh, PartitionSpec, NamedSharding
    from jax.experimental.shard_map import shard_map
    import concourse.bass2jax as b2j

    key = (id(nc), tuple(sorted((alias_out_to or {}).items())),
           tuple(donate_inputs))
    if key in _EXEC_CACHE:
        return _EXEC_CACHE[key]

    b2j.install_neuronx_cc_hook()
    partition_name = (nc.partition_id_tensor.name
                      if nc.partition_id_tensor else None)
    in_names, out_names, out_avals = [], [], []
    for alloc in nc.m.functions[0].allocations:
        if not isinstance(alloc, mybir.MemoryLocationSet):
            continue
        name = alloc.memorylocations[0].name
        if alloc.kind == "ExternalInput":
            if name != partition_name:
                in_names.append(name)
        elif alloc.kind == "ExternalOutput":
            out_names.append(name)
            shape = tuple(alloc.tensor_shape)
            dtype = mybir.dt.np(alloc.dtype)
            out_avals.append(jax.core.ShapedArray(shape, dtype))
    aliases = tuple(
        (out_names.index(o), in_names.index(i))
        for o, i in (alias_out_to or {}).items()
    )
    donate = tuple(in_names.index(i) for i in donate_inputs)
    all_in_names = list(in_names)
    if partition_name is not None:
        all_in_names.append(partition_name)

    def _body(*args):
        operands = list(args)
        if partition_name is not None:
            operands.append(b2j.partition_id_tensor())
        outs = b2j._bass_exec_p.bind(
            *operands,
            out_avals=tuple(out_avals),
            in_names=tuple(all_in_names),
            out_names=tuple(out_names),
            lowering_input_output_aliases=aliases,
            sim_require_finite=True,
            sim_require_nnan=True,
            nc=nc,
        )
        return tuple(outs)

    devices = jax.devices()[:NCORES]
    mesh = Mesh(np.asarray(devices), ("core",))
    spec = PartitionSpec("core")
    sharded = jax.jit(
        shard_map(_body, mesh=mesh,
                  in_specs=(spec,) * len(in_names),
                  out_specs=(spec,) * len(out_names),
                  check_rep=False),
        donate_argnums=donate,
        keep_unused=True,
    )
    sharding = NamedSharding(mesh, spec)
    entry = (sharded, in_names, out_names, out_avals, sharding)
    _EXEC_CACHE[key] = entry
    return entry


def _exec_spmd(nc, in_maps, alias_out_to=None, donate_inputs=()):
    """Run a compiled Bacc program on NCORES cores via the bass2jax PJRT
    path.  Returns list (per core) of {output_name: np.ndarray}."""
    import jax

    sharded, in_names, out_names, out_avals, sharding = _get_exec(
        nc, alias_out_to, donate_inputs)
    concat_in = [
        jax.device_put(
            np.concatenate([np.asarray(in_maps[c][nm]) for c in range(NCORES)],
                           axis=0), sharding)
        for nm in in_names
    ]
    out_arrs = sharded(*concat_in)
    jax.block_until_ready(out_arrs)
    return [
        {
            name: np.asarray(out_arrs[i]).reshape(NCORES, *out_avals[i].shape)[c]
            for i, name in enumerate(out_names)
        }
        for c in range(NCORES)
    ]


def _expected_updates(logits_flat, tokens):
    """Exact expected values at every penalized element: list of
    (row, col, expected_value) computed on host from the tiny token tensor."""
    tokens = np.asarray(tokens).astype(np.int64)
    updates = []
    for g in range(B * S):
        b, i = divmod(g, S)
        if i < M:
            continue
        cols, cnts = np.unique(tokens[b, i - M:i], return_counts=True)
        for col, n in zip(cols, cnts):
            updates.append((g, int(col),
                            logits_flat[g, int(col)]
                            - np.float32(0.3) * np.float32(n)))
    return updates


def kernel(logits, generated_tokens):
    global _INPLACE_OK
    logits = np.ascontiguousarray(np.asarray(logits, dtype=np.float32))
    pay_all, idx_all = _preprocess(generated_tokens)
    flat = logits.reshape(B * S, V)
    in_maps = [
        {
            "x": np.ascontiguousarray(flat[c * R:(c + 1) * R]).reshape(N),
            "pay": pay_all[c],
            "idx": idx_all[c],
        }
        for c in range(NCORES)
    ]

    if _INPLACE_OK is not False:
        res = _exec_spmd(_get_nc_inplace(1), in_maps,
                         alias_out_to={"out": "x"}, donate_inputs=("x",))
        out = np.concatenate([res[c]["out"] for c in range(NCORES)])
        out = out.reshape(B * S, V)
        if _INPLACE_OK is None:
            # Validate the aliasing path once: every penalized element must
            # match the host-exact value, and untouched elements must equal
            # the input.
            ok = True
            for g, col, exp in _expected_updates(flat, generated_tokens):
                if out[g, col] != exp:
                    ok = False
                    break
            if ok:
                rng = np.random.default_rng(0)
                gs = rng.integers(0, B * S, 2048)
                cs = rng.integers(0, V, 2048)
                touched = {(g, c) for g, c, _ in
                           _expected_updates(flat, generated_tokens)}
                for g, c in zip(gs, cs):
                    if (int(g), int(c)) in touched:
                        continue
                    if out[int(g), int(c)] != flat[int(g), int(c)]:
                        ok = False
                        break
            _INPLACE_OK = bool(ok)
            if not ok:
                # Aliasing not honored in this environment; fall back.
                return kernel(logits, generated_tokens)
        return out.reshape(B, S, V)

    from concourse.bass_utils import run_bass_kernel_spmd
    res = run_bass_kernel_spmd(_get_nc_copy(1), in_maps,
                               core_ids=list(range(NCORES)))
    out = np.concatenate([res.results[c]["out"] for c in range(NCORES)])
    return out.reshape(B, S, V)
